# revision 2
# baseline (speedup 1.0000x reference)
"""Bahdanau additive attention on Trainium2 (Bass/Tile), SPMD over 8 NeuronCores.

Sharding: sequence-parallel over tgt_len T - core i handles query rows
[i*32, (i+1)*32) for ALL batches; weights and encoder outputs replicated,
so every core does identical-size work regardless of src_lengths.

Design (ACT-bound):
- Per batch, per 16-row sweep: enc_fT[h,s] + qry_fT[h,t] is ONE DVE
  tensor_tensor using stride-0 (broadcast) access patterns over a
  (t, s, c)-interleaved bf16 layout; tanh is one big in-place ACT
  instruction over the whole sweep.
- Score dot with v: per query row, 4 chunk-matmuls accumulate in PSUM
  (c-loop inner: a PSUM zero region can hold only one open group);
  tile_position packs 16 rows into 2 banks.
- Softmax skips the max-subtraction (|score| <= ||v||_1 ~ 18 so exp is
  f32-safe); exp runs per-batch on [32, S] with accum_out giving row sums
  for free; weights are normalized before the attention matmul.
- Software pipeline: phase A (load/transpose/enc_f/qry_f) runs two batches
  ahead; batches are ordered shortest-first (fast ramp) / short-last
  (short drain); the first/last batches use finer add+tanh grains.
- src_lengths are read at trace time: loop extents are specialized, masked
  source positions are never computed.
"""

import numpy as np

NCORES = 8
P = 128

# add implementation: "merged" = 1 instr/sweep, "per_t" = 16 instr/sweep
ADD_MODE = "merged"
TANH_INPLACE = True


def _build_program(B, T_core, S, H, L, Lh, reps=1):
    import concourse.bass as bass  # noqa: F401
    import concourse.mybir as mybir
    import concourse.tile as tile
    from concourse import bacc
    from concourse.masks import make_identity

    f32 = mybir.dt.float32
    f32r = mybir.dt.float32r
    bf16 = mybir.dt.bfloat16
    AF = mybir.ActivationFunctionType

    HC = H // P  # 4 h-chunks

    nc = bacc.Bacc("TRN2", target_bir_lowering=False, debug=False)

    enc_d = nc.declare_dram_parameter("enc", [B, S, H], f32, isOutput=False)
    q_d = nc.declare_dram_parameter("q", [B, T_core, H], f32, isOutput=False)
    wh_d = nc.declare_dram_parameter("wh", [H, H], f32, isOutput=False)
    ws_d = nc.declare_dram_parameter("ws", [H, H], f32, isOutput=False)
    v_d = nc.declare_dram_parameter("v", [H], f32, isOutput=False)
    out_d = nc.declare_dram_parameter("out", [B, T_core, H], f32, isOutput=True)

    def r32(ap):
        return ap.bitcast(f32r)

    # batch processing order: shortest first (fast ramp to the first tanh),
    # then second-shortest (pipeline still filling), then longest-first so
    # the drain tail lands on a short batch.
    asc = sorted(range(B), key=lambda b: Lh[b])
    border = [asc[0], asc[2]] + sorted(asc[3:], key=lambda b: -Lh[b]) + [asc[1]]

    with tile.TileContext(nc) as tc:
        with (
            tc.tile_pool(name="const", bufs=1) as constp,
            tc.tile_pool(name="sb", bufs=2) as sb,
            tc.tile_pool(name="work", bufs=2) as workp,
            tc.tile_pool(name="ps", bufs=2, space="PSUM") as psp,
            tc.tile_pool(name="ps_sc", bufs=1, space="PSUM") as pssc,
        ):
            ident_f = constp.tile([P, P], f32)
            make_identity(nc, ident_f)

            # v -> [128, HC] f32; v32 block c = [v chunk c, 0 x 31] bf16
            v_f = constp.tile([P, HC], f32)
            nc.gpsimd.dma_start(v_f, v_d.rearrange("(c p) -> p c", p=P))
            v32 = constp.tile([P, HC * 32], bf16)
            nc.vector.memset(v32, 0.0)
            for c in range(HC):
                nc.vector.tensor_copy(v32[:, c * 32 : c * 32 + 1], v_f[:, c : c + 1])

            # W_h^T bf16, W_s^T f32 : block k = W^T[h' in chunk k, :]
            # (weight DMAs issued from GPSIMD so they don't serialize behind
            # the first batch's enc/q loads on SP)
            whT = constp.tile([P, HC * H], bf16)
            wsT = constp.tile([P, HC * H], f32)
            whn = []
            wsn = []
            # spread the 8 weight-block DMAs over 2 extra issuing engines so
            # the transfers overlap instead of serializing on one DGE queue
            dma_eng = [nc.gpsimd, nc.scalar, nc.gpsimd, nc.scalar]
            for c in range(HC):
                wn = sb.tile([P, H], f32, name=f"whn{c}", tag="wn", bufs=4)
                dma_eng[c].dma_start(wn, wh_d[c * P : (c + 1) * P, :])
                whn.append(wn)
                wsn_c = sb.tile([P, H], f32, name=f"wsn{c}", tag="wsn", bufs=4)
                dma_eng[(c + 1) % 4].dma_start(wsn_c, ws_d[c * P : (c + 1) * P, :])
                wsn.append(wsn_c)
            ident_b = constp.tile([P, P], bf16)
            make_identity(nc, ident_b)
            # psum->sbuf weight copies run on ACT (idle during startup);
            # the f32->bf16 narrowing for whT happens in the copy.
            for k in range(HC):
                pst = psp.tile([P, HC * P], f32, name=f"whT_ps{k}", tag="mmC", bufs=2)
                for c in range(HC):
                    nc.tensor.transpose(
                        pst[:, c * P : (c + 1) * P],
                        whn[c][:, k * P : (k + 1) * P],
                        ident_f,
                    )
                nc.scalar.copy(whT[:, k * H : (k + 1) * H], pst)
            for k in range(HC):
                psf = psp.tile([P, HC * P], f32, name=f"wsT_ps{k}", tag="mmC", bufs=2)
                for c in range(HC):
                    nc.tensor.transpose(
                        psf[:, c * P : (c + 1) * P],
                        wsn[c][:, k * P : (k + 1) * P],
                        ident_f,
                    )
                nc.scalar.copy(wsT[:, k * H : (k + 1) * H], psf)

            def load(b):
                Lhb = Lh[b]
                nk = (Lhb + P - 1) // P
                enc_nat = []
                for k2 in range(nk):
                    r2 = min(P, Lhb - k2 * P)
                    en = sb.tile([P, H], f32, name=f"enc{b}_{k2}", tag=f"enc{k2}", bufs=3)
                    nc.sync.dma_start(en[:r2, :], enc_d[b, k2 * P : k2 * P + r2, :])
                    enc_nat.append((en, r2))
                qn = sb.tile([T_core, H], f32, name=f"qn{b}", tag="qn", bufs=3)
                nc.sync.dma_start(qn, q_d[b])
                return enc_nat, qn

            def phase_a(b, enc_nat, qn):
                """encfT_i bf16 (s,c)-interleaved, qfT_i bf16 (t,c)-interleaved."""
                Lhb = Lh[b]
                nk = (Lhb + P - 1) // P
                # encT (bf16): block k = enc^T[h' in chunk k, s]
                encT = sb.tile([P, HC * S], bf16, name=f"encT{b}", tag="encT", bufs=3)
                encT_v = encT.rearrange("p (k s) -> p k s", k=HC)
                for k2 in range(nk):
                    en, r2 = enc_nat[k2]
                    ps_t = psp.tile(
                        [P, HC * P], f32, name=f"encT_ps{b}_{k2}", tag="mmA", bufs=1
                    )
                    for k in range(HC):
                        nc.tensor.transpose(
                            ps_t[:, k * P : k * P + r2],
                            en[:r2, k * P : (k + 1) * P],
                            ident_f[:r2, :r2],
                        )
                    nc.vector.tensor_copy(
                        encT_v[:, :, k2 * P : k2 * P + r2],
                        ps_t.rearrange("p (k s) -> p k s", k=HC)[:, :, :r2],
                    )
                # enc_fT = W_h @ enc^T -> psum [P, (c-block, s)]
                ps_e = psp.tile([P, HC * S], f32, name=f"encf_ps{b}", tag="mmA", bufs=1)
                for c in range(HC):
                    for k in range(HC):
                        nc.tensor.matmul(
                            ps_e[:, c * S : c * S + Lhb],
                            whT[:, k * H + c * P : k * H + (c + 1) * P],
                            encT_v[:, k, :Lhb],
                            start=(k == 0),
                            stop=(k == HC - 1),
                        )
                # copy to bf16 INTERLEAVED: col s*HC + c
                encfT_i = sb.tile(
                    [P, HC * S], bf16, name=f"encfT{b}", tag="encfT", bufs=3
                )
                nc.vector.tensor_copy(
                    encfT_i.rearrange("p (s c) -> p c s", c=HC)[:, :, :Lhb],
                    ps_e.rearrange("p (c s) -> p c s", c=HC)[:, :, :Lhb],
                )
                # qry_fT: transpose q, W_s @ q^T (f32), then bf16 interleave
                ps_q = psp.tile(
                    [P, HC * T_core], f32, name=f"qT_ps{b}", tag="mmA", bufs=1
                )
                for k in range(HC):
                    nc.tensor.transpose(
                        ps_q[:, k * T_core : (k + 1) * T_core],
                        qn[:, k * P : (k + 1) * P],
                        ident_f[:T_core, :T_core],
                    )
                qT = sb.tile([P, HC * T_core], f32, name=f"qT{b}", tag="qT", bufs=3)
                nc.vector.tensor_copy(qT, ps_q)
                ps_qf = psp.tile(
                    [P, HC * T_core], f32, name=f"qf_ps{b}", tag="mmA", bufs=1
                )
                for c in range(HC):
                    for k in range(HC):
                        nc.tensor.matmul(
                            ps_qf[:, c * T_core : (c + 1) * T_core],
                            wsT[:, k * H + c * P : k * H + (c + 1) * P],
                            qT[:, k * T_core : (k + 1) * T_core],
                            start=(k == 0),
                            stop=(k == HC - 1),
                        )
                qfT_i = sb.tile(
                    [P, T_core * HC], bf16, name=f"qfT{b}", tag="qfT", bufs=3
                )
                nc.vector.tensor_copy(
                    qfT_i.rearrange("p (t c) -> p c t", c=HC),
                    ps_qf.rearrange("p (c t) -> p c t", c=HC),
                )
                return encfT_i, qfT_i

            def phase_bc(b, enc_nat, encfT_i, qfT_i, split=1):
                Lb, Lhb = L[b], Lh[b]
                nk = (Lhb + P - 1) // P
                n_sweeps = T_core // 16
                G = 16 // split  # query rows per add/tanh instruction

                sc_b = sb.tile([T_core, S], f32, name=f"sc{b}", tag="scsb")
                for sweep in range(n_sweeps):
                    ps_scores = pssc.tile(
                        [P, 2 * 512], f32, name=f"sc_ps{b}_{sweep}",
                        tag="scores", bufs=2,
                    )
                    for g in range(split):
                        t0 = g * G  # first row of this group within the sweep
                        # ---- add + tanh over G query rows ----
                        st = workp.tile(
                            [P, G * HC * Lhb], bf16, name=f"st{b}_{sweep}_{g}",
                            tag="st", bufs=3,
                        )
                        st_v = st.rearrange("p (t s c) -> p t s c", t=G, c=HC)
                        in0 = (
                            encfT_i[:, : HC * Lhb]
                            .rearrange("p (s c) -> p s c", c=HC)
                            .unsqueeze(1)
                            .broadcast_to((P, G, Lhb, HC))
                        )
                        q0 = (sweep * 16 + t0) * HC
                        in1 = (
                            qfT_i[:, q0 : q0 + G * HC]
                            .rearrange("p (t c) -> p t c", c=HC)
                            .unsqueeze(2)
                            .broadcast_to((P, G, Lhb, HC))
                        )
                        nc.vector.tensor_tensor(
                            st_v, in0, in1, op=mybir.AluOpType.add
                        )
                        nc.scalar.activation(st, st, AF.Tanh)
                        tanh_v = st.rearrange("p (t s c) -> p t s c", t=G, c=HC)

                        # ---- scores: per query row, 4 consecutive chunk-
                        # matmuls; a PSUM zero region can hold only one open
                        # accumulation group, so the c-loop must be inner ----
                        for tl in range(G):
                            tt = t0 + tl
                            cg, m = tt // 4, tt % 4
                            for c in range(HC):
                                nc.tensor.matmul(
                                    ps_scores[
                                        32 * cg : 32 * cg + 32,
                                        256 * m : 256 * m + Lhb,
                                    ],
                                    v32[:, c * 32 : (c + 1) * 32],
                                    tanh_v[:, tl, :, c],
                                    start=(c == 0),
                                    stop=(c == HC - 1),
                                    tile_position=(0, 32 * cg),
                                )
                    # ---- PSUM -> SBUF staging, gather to [16, S] rows ----
                    stage = sb.tile(
                        [P, 4 * 256], f32, name=f"stage{b}_{sweep}", tag="stage"
                    )
                    nc.vector.tensor_copy(
                        stage.rearrange("p (m s) -> p m s", m=4)[:, :, :Lb],
                        ps_scores.rearrange("p (m s) -> p m s", m=4)[:, :, :Lb],
                    )
                    src = stage.rearrange("(a p) (m s) -> a p m s", a=4, m=4)[
                        :, 0, :, :Lb
                    ]
                    nc.sync.dma_start(
                        sc_b[sweep * 16 : (sweep + 1) * 16, :Lb], src
                    )

                # ---- softmax (no max subtraction; |score| <= ||v||_1) ----
                w_b = sb.tile([T_core, S], f32, name=f"w{b}", tag="w")
                if Lb < S:
                    nc.vector.memset(w_b[:, Lb:], 0.0)
                sums = sb.tile([T_core, 1], f32, name=f"sums{b}", tag="sums")
                nc.scalar.activation(
                    w_b[:, :Lb], sc_b[:, :Lb], AF.Exp, accum_out=sums
                )
                recip = sb.tile([T_core, 1], f32, name=f"recip{b}", tag="recip")
                nc.vector.reciprocal(recip, sums)
                nc.vector.tensor_scalar_mul(w_b[:, :Lb], w_b[:, :Lb], recip)

                # ---- attn = w_norm @ enc  (f32r moving, 1 cyc/row) ----
                ps_w = psp.tile(
                    [P, 2 * T_core], f32, name=f"wT_ps{b}", tag="mmC", bufs=2
                )
                for k2 in range(nk):
                    nc.tensor.transpose(
                        ps_w[:, k2 * T_core : (k2 + 1) * T_core],
                        w_b[:, k2 * P : (k2 + 1) * P],
                        ident_f[:T_core, :T_core],
                    )
                wT = sb.tile([P, 2 * T_core], f32, name=f"wT{b}", tag="wT")
                nc.vector.tensor_copy(wT[:, : nk * T_core], ps_w[:, : nk * T_core])
                ps_attn = psp.tile(
                    [T_core, H], f32, name=f"attn_ps{b}", tag="mmC", bufs=2
                )
                for k2 in range(nk):
                    en, r2 = enc_nat[k2]
                    nc.tensor.matmul(
                        ps_attn,
                        wT[:r2, k2 * T_core : (k2 + 1) * T_core],
                        en[:r2, :],
                        start=(k2 == 0),
                        stop=(k2 == nk - 1),
                    )
                out_sb = sb.tile([T_core, H], f32, name=f"out{b}", tag="outsb")
                nc.vector.tensor_copy(out_sb, ps_attn)
                nc.sync.dma_start(out_d[b], out_sb)

            def batch_loop():
                # software pipeline, depth 2: phase A of batches i+1 and i+2
                # is emitted before phase B/C of batch i so the DVE add of
                # the next batch is always ready when ACT finishes a tanh.
                pipe = []
                for b in border[:2]:
                    st = load(b)
                    pipe.append((b, st, phase_a(b, *st)))
                for i in range(B):
                    if i + 2 < B:
                        b2 = border[i + 2]
                        st2 = load(b2)
                        pipe.append((b2, st2, phase_a(b2, *st2)))
                    b, st, pa = pipe.pop(0)
                    # fine-grained add/tanh on the first batch (ACT ramps
                    # sooner) and the last (shorter drain tail)
                    split = 4 if i == 0 else (2 if i == B - 1 else 1)
                    phase_bc(b, st[0], *pa, split=split)

            if reps > 1:
                with tc.For_i(0, reps, 1):
                    batch_loop()
            else:
                batch_loop()

    nc.compile()
    return nc


LAST_EXEC_NS = None


def _get_program(key):
    B, T_core, S, H, L, Lh = key
    return _build_program(B, T_core, S, H, list(L), list(Lh))


def kernel(query, encoder_outputs, src_lengths, W_h, W_s, v):
    global LAST_EXEC_NS
    from concourse.bass_utils import run_bass_kernel_spmd

    query = np.ascontiguousarray(np.asarray(query, dtype=np.float32))
    enc = np.ascontiguousarray(np.asarray(encoder_outputs, dtype=np.float32))
    W_h = np.ascontiguousarray(np.asarray(W_h, dtype=np.float32))
    W_s = np.ascontiguousarray(np.asarray(W_s, dtype=np.float32))
    v = np.ascontiguousarray(np.asarray(v, dtype=np.float32)).reshape(-1)
    L = [int(x) for x in np.asarray(src_lengths).reshape(-1)]

    B, T, H = query.shape
    S = enc.shape[1]
    T_core = T // NCORES
    Lh = [min(S, ((l + 3) // 4) * 4) for l in L]

    nc = _get_program((B, T_core, S, H, tuple(L), tuple(Lh)))

    in_maps = [
        {
            "enc": enc,
            "q": np.ascontiguousarray(query[:, i * T_core : (i + 1) * T_core, :]),
            "wh": W_h,
            "ws": W_s,
            "v": v,
        }
        for i in range(NCORES)
    ]
    res = run_bass_kernel_spmd(nc, in_maps, list(range(NCORES)))
    LAST_EXEC_NS = res.exec_time_ns
    out = np.concatenate([res.results[i]["out"] for i in range(NCORES)], axis=1)
    return out


# revision 4
# speedup vs baseline: 2.0540x; 2.0540x over previous
"""Bahdanau additive attention on Trainium2 (Bass/Tile), SPMD over 8 NeuronCores.

Sharding: sequence-parallel over tgt_len T - core i handles query rows
[i*32, (i+1)*32) for ALL batches; weights and encoder outputs replicated,
so every core does identical-size work regardless of src_lengths.

Design (ACT-bound):
- Per batch, per 16-row sweep: enc_fT[h,s] + qry_fT[h,t] is ONE DVE
  tensor_tensor using stride-0 (broadcast) access patterns over a
  (t, s, c)-interleaved bf16 layout; tanh is one big in-place ACT
  instruction over the whole sweep.
- Score dot with v: per query row, 4 chunk-matmuls accumulate in PSUM
  (c-loop inner: a PSUM zero region can hold only one open group);
  tile_position packs 16 rows into 2 banks.
- Softmax skips the max-subtraction (|score| <= ||v||_1 ~ 18 so exp is
  f32-safe); exp runs per-batch on [32, S] with accum_out giving row sums
  for free; weights are normalized before the attention matmul.
- Software pipeline: phase A (load/transpose/enc_f/qry_f) runs two batches
  ahead; batches are ordered shortest-first (fast ramp) / short-last
  (short drain); the first/last batches use finer add+tanh grains.
- src_lengths are read at trace time: loop extents are specialized, masked
  source positions are never computed.
"""

import numpy as np

NCORES = 8
P = 128


def _build_program(B, T_core, S, H, L, Lh, reps=1):
    import concourse.bass as bass  # noqa: F401
    import concourse.mybir as mybir
    import concourse.tile as tile
    from concourse import bacc
    from concourse.masks import make_identity

    f32 = mybir.dt.float32
    bf16 = mybir.dt.bfloat16
    AF = mybir.ActivationFunctionType

    HC = H // P  # 4 h-chunks

    nc = bacc.Bacc("TRN2", target_bir_lowering=False, debug=False)

    enc_d = nc.declare_dram_parameter("enc", [B, S, H], f32, isOutput=False)
    q_d = nc.declare_dram_parameter("q", [B, T_core, H], f32, isOutput=False)
    wh_d = nc.declare_dram_parameter("wh", [H, H], f32, isOutput=False)
    ws_d = nc.declare_dram_parameter("ws", [H, H], f32, isOutput=False)
    v_d = nc.declare_dram_parameter("v", [H], f32, isOutput=False)
    out_d = nc.declare_dram_parameter("out", [B, T_core, H], f32, isOutput=True)

    # batch processing order: shortest first (fast ramp to the first tanh),
    # then second-shortest (pipeline still filling), then longest-first so
    # the drain tail lands on a short batch.
    asc = sorted(range(B), key=lambda b: Lh[b])
    if B >= 3:
        border = (
            [asc[0], asc[2]]
            + sorted(asc[3:], key=lambda b: -Lh[b])
            + [asc[1]]
        )
    else:
        border = asc

    with tile.TileContext(nc) as tc:
        with (
            tc.tile_pool(name="const", bufs=1) as constp,
            tc.tile_pool(name="sb", bufs=2) as sb,
            tc.tile_pool(name="work", bufs=2) as workp,
            tc.tile_pool(name="ps", bufs=2, space="PSUM") as psp,
            tc.tile_pool(name="ps_sc", bufs=1, space="PSUM") as pssc,
        ):
            ident_f = constp.tile([P, P], f32)
            make_identity(nc, ident_f)

            # v -> [128, HC] f32; v32 block c = [v chunk c, 0 x 31] bf16
            v_f = constp.tile([P, HC], f32)
            nc.gpsimd.dma_start(v_f, v_d.rearrange("(c p) -> p c", p=P))
            v32 = constp.tile([P, HC * 32], bf16)
            nc.vector.memset(v32, 0.0)
            for c in range(HC):
                nc.vector.tensor_copy(v32[:, c * 32 : c * 32 + 1], v_f[:, c : c + 1])

            # W_h^T bf16, W_s^T f32 : block k = W^T[h' in chunk k, :]
            # (weight DMAs issued from GPSIMD so they don't serialize behind
            # the first batch's enc/q loads on SP)
            whT = constp.tile([P, HC * H], bf16)
            wsT = constp.tile([P, HC * H], f32)
            whn = []
            wsn = []
            # spread the 8 weight-block DMAs over 2 extra issuing engines so
            # the transfers overlap instead of serializing on one DGE queue
            dma_eng = [nc.gpsimd, nc.scalar, nc.gpsimd, nc.scalar]
            for c in range(HC):
                wn = sb.tile([P, H], f32, name=f"whn{c}", tag="wn", bufs=4)
                dma_eng[c].dma_start(wn, wh_d[c * P : (c + 1) * P, :])
                whn.append(wn)
                wsn_c = sb.tile([P, H], f32, name=f"wsn{c}", tag="wsn", bufs=4)
                dma_eng[(c + 1) % 4].dma_start(wsn_c, ws_d[c * P : (c + 1) * P, :])
                wsn.append(wsn_c)
            # psum->sbuf weight copies run on ACT (idle during startup);
            # the f32->bf16 narrowing for whT happens in the copy.
            for k in range(HC):
                pst = psp.tile([P, HC * P], f32, name=f"whT_ps{k}", tag="mmC", bufs=2)
                for c in range(HC):
                    nc.tensor.transpose(
                        pst[:, c * P : (c + 1) * P],
                        whn[c][:, k * P : (k + 1) * P],
                        ident_f,
                    )
                nc.scalar.copy(whT[:, k * H : (k + 1) * H], pst)
            for k in range(HC):
                psf = psp.tile([P, HC * P], f32, name=f"wsT_ps{k}", tag="mmC", bufs=2)
                for c in range(HC):
                    nc.tensor.transpose(
                        psf[:, c * P : (c + 1) * P],
                        wsn[c][:, k * P : (k + 1) * P],
                        ident_f,
                    )
                nc.scalar.copy(wsT[:, k * H : (k + 1) * H], psf)

            def load(b):
                Lhb = Lh[b]
                nk = (Lhb + P - 1) // P
                enc_nat = []
                for k2 in range(nk):
                    r2 = min(P, Lhb - k2 * P)
                    en = sb.tile([P, H], f32, name=f"enc{b}_{k2}", tag=f"enc{k2}", bufs=3)
                    nc.sync.dma_start(en[:r2, :], enc_d[b, k2 * P : k2 * P + r2, :])
                    enc_nat.append((en, r2))
                qn = sb.tile([T_core, H], f32, name=f"qn{b}", tag="qn", bufs=3)
                nc.sync.dma_start(qn, q_d[b])
                return enc_nat, qn

            def phase_a(b, enc_nat, qn):
                """encfT_i bf16 (s,c)-interleaved, qfT_i bf16 (t,c)-interleaved."""
                Lhb = Lh[b]
                nk = (Lhb + P - 1) // P
                # encT (bf16): block k = enc^T[h' in chunk k, s]
                encT = sb.tile([P, HC * S], bf16, name=f"encT{b}", tag="encT", bufs=3)
                encT_v = encT.rearrange("p (k s) -> p k s", k=HC)
                for k2 in range(nk):
                    en, r2 = enc_nat[k2]
                    ps_t = psp.tile(
                        [P, HC * P], f32, name=f"encT_ps{b}_{k2}", tag="mmA", bufs=1
                    )
                    for k in range(HC):
                        nc.tensor.transpose(
                            ps_t[:, k * P : k * P + r2],
                            en[:r2, k * P : (k + 1) * P],
                            ident_f[:r2, :r2],
                        )
                    nc.vector.tensor_copy(
                        encT_v[:, :, k2 * P : k2 * P + r2],
                        ps_t.rearrange("p (k s) -> p k s", k=HC)[:, :, :r2],
                    )
                # enc_fT = W_h @ enc^T -> psum [P, (c-block, s)]
                ps_e = psp.tile([P, HC * S], f32, name=f"encf_ps{b}", tag="mmA", bufs=1)
                for c in range(HC):
                    for k in range(HC):
                        nc.tensor.matmul(
                            ps_e[:, c * S : c * S + Lhb],
                            whT[:, k * H + c * P : k * H + (c + 1) * P],
                            encT_v[:, k, :Lhb],
                            start=(k == 0),
                            stop=(k == HC - 1),
                        )
                # copy to bf16 INTERLEAVED: col s*HC + c
                encfT_i = sb.tile(
                    [P, HC * S], bf16, name=f"encfT{b}", tag="encfT", bufs=3
                )
                nc.vector.tensor_copy(
                    encfT_i.rearrange("p (s c) -> p c s", c=HC)[:, :, :Lhb],
                    ps_e.rearrange("p (c s) -> p c s", c=HC)[:, :, :Lhb],
                )
                # qry_fT: transpose q, W_s @ q^T (f32), then bf16 interleave
                ps_q = psp.tile(
                    [P, HC * T_core], f32, name=f"qT_ps{b}", tag="mmA", bufs=1
                )
                for k in range(HC):
                    nc.tensor.transpose(
                        ps_q[:, k * T_core : (k + 1) * T_core],
                        qn[:, k * P : (k + 1) * P],
                        ident_f[:T_core, :T_core],
                    )
                qT = sb.tile([P, HC * T_core], f32, name=f"qT{b}", tag="qT", bufs=3)
                nc.vector.tensor_copy(qT, ps_q)
                ps_qf = psp.tile(
                    [P, HC * T_core], f32, name=f"qf_ps{b}", tag="mmA", bufs=1
                )
                for c in range(HC):
                    for k in range(HC):
                        nc.tensor.matmul(
                            ps_qf[:, c * T_core : (c + 1) * T_core],
                            wsT[:, k * H + c * P : k * H + (c + 1) * P],
                            qT[:, k * T_core : (k + 1) * T_core],
                            start=(k == 0),
                            stop=(k == HC - 1),
                        )
                qfT_i = sb.tile(
                    [P, T_core * HC], bf16, name=f"qfT{b}", tag="qfT", bufs=3
                )
                nc.vector.tensor_copy(
                    qfT_i.rearrange("p (t c) -> p c t", c=HC),
                    ps_qf.rearrange("p (c t) -> p c t", c=HC),
                )
                return encfT_i, qfT_i

            def phase_bc(b, enc_nat, encfT_i, qfT_i, split=1):
                Lb, Lhb = L[b], Lh[b]
                nk = (Lhb + P - 1) // P
                n_sweeps = T_core // 16
                G = 16 // split  # query rows per add/tanh instruction

                sc_b = sb.tile([T_core, S], f32, name=f"sc{b}", tag="scsb")
                for sweep in range(n_sweeps):
                    ps_scores = pssc.tile(
                        [P, 2 * 512], f32, name=f"sc_ps{b}_{sweep}",
                        tag="scores", bufs=2,
                    )
                    for g in range(split):
                        t0 = g * G  # first row of this group within the sweep
                        # ---- add + tanh over G query rows ----
                        st = workp.tile(
                            [P, G * HC * Lhb], bf16, name=f"st{b}_{sweep}_{g}",
                            tag="st", bufs=3,
                        )
                        st_v = st.rearrange("p (t s c) -> p t s c", t=G, c=HC)
                        in0 = (
                            encfT_i[:, : HC * Lhb]
                            .rearrange("p (s c) -> p s c", c=HC)
                            .unsqueeze(1)
                            .broadcast_to((P, G, Lhb, HC))
                        )
                        q0 = (sweep * 16 + t0) * HC
                        in1 = (
                            qfT_i[:, q0 : q0 + G * HC]
                            .rearrange("p (t c) -> p t c", c=HC)
                            .unsqueeze(2)
                            .broadcast_to((P, G, Lhb, HC))
                        )
                        nc.vector.tensor_tensor(
                            st_v, in0, in1, op=mybir.AluOpType.add
                        )
                        nc.scalar.activation(st, st, AF.Tanh)
                        tanh_v = st.rearrange("p (t s c) -> p t s c", t=G, c=HC)

                        # ---- scores: per query row, 4 consecutive chunk-
                        # matmuls; a PSUM zero region can hold only one open
                        # accumulation group, so the c-loop must be inner ----
                        for tl in range(G):
                            tt = t0 + tl
                            cg, m = tt // 4, tt % 4
                            for c in range(HC):
                                nc.tensor.matmul(
                                    ps_scores[
                                        32 * cg : 32 * cg + 32,
                                        256 * m : 256 * m + Lhb,
                                    ],
                                    v32[:, c * 32 : (c + 1) * 32],
                                    tanh_v[:, tl, :, c],
                                    start=(c == 0),
                                    stop=(c == HC - 1),
                                    tile_position=(0, 32 * cg),
                                )
                    # ---- PSUM -> SBUF staging, gather to [16, S] rows ----
                    stage = sb.tile(
                        [P, 4 * 256], f32, name=f"stage{b}_{sweep}", tag="stage"
                    )
                    nc.vector.tensor_copy(
                        stage.rearrange("p (m s) -> p m s", m=4)[:, :, :Lb],
                        ps_scores.rearrange("p (m s) -> p m s", m=4)[:, :, :Lb],
                    )
                    src = stage.rearrange("(a p) (m s) -> a p m s", a=4, m=4)[
                        :, 0, :, :Lb
                    ]
                    nc.sync.dma_start(
                        sc_b[sweep * 16 : (sweep + 1) * 16, :Lb], src
                    )

                # ---- softmax (no max subtraction; |score| <= ||v||_1) ----
                w_b = sb.tile([T_core, S], f32, name=f"w{b}", tag="w")
                if Lb < S:
                    nc.vector.memset(w_b[:, Lb:], 0.0)
                sums = sb.tile([T_core, 1], f32, name=f"sums{b}", tag="sums")
                nc.scalar.activation(
                    w_b[:, :Lb], sc_b[:, :Lb], AF.Exp, accum_out=sums
                )
                recip = sb.tile([T_core, 1], f32, name=f"recip{b}", tag="recip")
                nc.vector.reciprocal(recip, sums)
                nc.vector.tensor_scalar_mul(w_b[:, :Lb], w_b[:, :Lb], recip)

                # ---- attn = w_norm @ enc  (f32r moving, 1 cyc/row) ----
                ps_w = psp.tile(
                    [P, 2 * T_core], f32, name=f"wT_ps{b}", tag="mmC", bufs=2
                )
                for k2 in range(nk):
                    nc.tensor.transpose(
                        ps_w[:, k2 * T_core : (k2 + 1) * T_core],
                        w_b[:, k2 * P : (k2 + 1) * P],
                        ident_f[:T_core, :T_core],
                    )
                wT = sb.tile([P, 2 * T_core], f32, name=f"wT{b}", tag="wT")
                nc.vector.tensor_copy(wT[:, : nk * T_core], ps_w[:, : nk * T_core])
                ps_attn = psp.tile(
                    [T_core, H], f32, name=f"attn_ps{b}", tag="mmC", bufs=2
                )
                for k2 in range(nk):
                    en, r2 = enc_nat[k2]
                    nc.tensor.matmul(
                        ps_attn,
                        wT[:r2, k2 * T_core : (k2 + 1) * T_core],
                        en[:r2, :],
                        start=(k2 == 0),
                        stop=(k2 == nk - 1),
                    )
                out_sb = sb.tile([T_core, H], f32, name=f"out{b}", tag="outsb")
                nc.vector.tensor_copy(out_sb, ps_attn)
                nc.sync.dma_start(out_d[b], out_sb)

            def batch_loop():
                # software pipeline, depth 2: phase A of batches i+1 and i+2
                # is emitted before phase B/C of batch i so the DVE add of
                # the next batch is always ready when ACT finishes a tanh.
                pipe = []
                for b in border[:2]:
                    st = load(b)
                    pipe.append((b, st, phase_a(b, *st)))
                for i in range(B):
                    if i + 2 < B:
                        b2 = border[i + 2]
                        st2 = load(b2)
                        pipe.append((b2, st2, phase_a(b2, *st2)))
                    b, st, pa = pipe.pop(0)
                    # fine-grained add/tanh on the first batch (ACT ramps
                    # sooner) and the last (shorter drain tail)
                    split = 4 if i == 0 else (2 if i == B - 1 else 1)
                    phase_bc(b, st[0], *pa, split=split)

            if reps > 1:
                with tc.For_i(0, reps, 1):
                    batch_loop()
            else:
                batch_loop()

    nc.compile()
    return nc


LAST_EXEC_NS = None


def _get_program(key):
    B, T_core, S, H, L, Lh = key
    return _build_program(B, T_core, S, H, list(L), list(Lh))


def kernel(query, encoder_outputs, src_lengths, W_h, W_s, v):
    global LAST_EXEC_NS
    from concourse.bass_utils import run_bass_kernel_spmd

    query = np.ascontiguousarray(np.asarray(query, dtype=np.float32))
    enc = np.ascontiguousarray(np.asarray(encoder_outputs, dtype=np.float32))
    W_h = np.ascontiguousarray(np.asarray(W_h, dtype=np.float32))
    W_s = np.ascontiguousarray(np.asarray(W_s, dtype=np.float32))
    v = np.ascontiguousarray(np.asarray(v, dtype=np.float32)).reshape(-1)
    L = [int(x) for x in np.asarray(src_lengths).reshape(-1)]

    B, T, H = query.shape
    S = enc.shape[1]
    T_core = T // NCORES
    Lh = [min(S, ((l + 3) // 4) * 4) for l in L]

    nc = _get_program((B, T_core, S, H, tuple(L), tuple(Lh)))

    in_maps = [
        {
            "enc": enc,
            "q": np.ascontiguousarray(query[:, i * T_core : (i + 1) * T_core, :]),
            "wh": W_h,
            "ws": W_s,
            "v": v,
        }
        for i in range(NCORES)
    ]
    res = run_bass_kernel_spmd(nc, in_maps, list(range(NCORES)))
    LAST_EXEC_NS = res.exec_time_ns
    out = np.concatenate([res.results[i]["out"] for i in range(NCORES)], axis=1)
    return out
